# revision 9
# baseline (speedup 1.0000x reference)
"""Trainium2 Bass kernel for AttentionWithRotaryPositionalEmbedding.

Problem shapes (hardcoded): x [4, 2048, 512], 8 heads, head dim 64.
Sharding: 8 cores = (batch b = core//2) x (query half = core%2).
Each core computes a [1024, 512] slice of the output; k/v are computed
locally from the full x[b] so no collectives are needed.

Device layout per core (fp32 everywhere):
  - host passes x[b]^T with the core's query half rotated to columns 0:1024
    (a pure permutation of the key axis; attention output is invariant).
  - qT/kT produced in [c, n] layout directly (c on partitions) so scores and
    attn@v need no transposes.
  - RoPE = t_c + M2 @ t_s where t_c/t_s are elementwise cos/sin products and
    M2 is the constant pairwise-rotation permutation (done on PE).
  - scores computed transposed: sT[k, q] = kT_h^T-style matmul; exp on ACT
    with fused *0.125; additive mask folded in exactly as exp(mask) scaling
    of v rows and of the ones-column that produces the softmax denominators.
  - attn@v: lhsT = [v_h | emask] (M=65) accumulating over k chunks; row 64 of
    the psum is the softmax denominator. Normalization is applied after, via
    a PE-replicated fast reciprocal, before the output projection.
"""

import sys

import numpy as np

if "/opt/trn_rl_repo" not in sys.path:
    sys.path.insert(0, "/opt/trn_rl_repo")

B, N, C = 4, 2048, 512
H, DH = 8, 64
NQ = 1024  # queries per core
P = 128
NCHUNK = N // P  # 16 k chunks
MAX_FPS = np.float32(30.0)

_CACHE = {}


def _host_prep(x, mask, times, Wqkv, Wproj, bproj):
    """Build per-core input maps (numpy only)."""
    x = np.asarray(x, np.float32)
    mask = np.asarray(mask, np.float32)
    times = np.asarray(times, np.float32)
    Wqkv = np.asarray(Wqkv, np.float32)
    Wproj = np.asarray(Wproj, np.float32)
    bproj = np.asarray(bproj, np.float32).reshape(1, C)

    wt = np.ascontiguousarray(Wqkv.T)          # [512, 1536] = [WqT|WkT|WvT]
    wpt = np.ascontiguousarray(Wproj.T)        # [512, 512]

    # pairwise rotation permutation: (M2 @ v)[2i] = -v[2i+1]; [2i+1] = +v[2i]
    M2 = np.zeros((P, P), np.float32)
    for i in range(P // 2):
        M2[2 * i, 2 * i + 1] = -1.0
        M2[2 * i + 1, 2 * i] = 1.0
    m2t = np.ascontiguousarray(M2.T)

    # rotary tables (match reference: all f32 math)
    inv_freq = (np.float32(1.0) /
                (np.float32(10000.0) **
                 (np.arange(0, DH, 2, dtype=np.float32) / np.float32(DH))))  # [32]
    pos = np.round(times * MAX_FPS)  # [B, N] f32, round-half-even like jnp

    in_maps = []
    for core in range(8):
        b, qhalf = core // 2, core % 2
        if qhalf == 0:
            perm = np.arange(N)
        else:
            perm = np.r_[NQ:N, 0:NQ]
        xt = np.ascontiguousarray(x[b].T[:, perm])           # [512, 2048]
        freqs = pos[b][perm][None, :] * inv_freq[:, None]     # [32, 2048] f32
        cos32 = np.cos(freqs.astype(np.float32))
        sin32 = np.sin(freqs.astype(np.float32))
        ridx = (np.arange(P) % DH) // 2                       # row -> pair index
        cose = np.ascontiguousarray(cos32[ridx])              # [128, 2048]
        sine = np.ascontiguousarray(sin32[ridx])
        em = np.exp(mask[b][perm]).astype(np.float32)         # [2048]
        emask = np.ascontiguousarray(em.reshape(NCHUNK, P).T) # [128, 16]
        in_maps.append({
            "xt": xt, "wt": wt, "wpt": wpt, "bias": bproj,
            "cose": cose, "sine": sine, "emask": emask, "m2t": m2t,
        })
    return in_maps


def _build_module():
    import concourse.bass as bass
    import concourse.tile as tile
    import concourse.mybir as mybir
    from concourse import bacc

    f32 = mybir.dt.float32
    nc = bacc.Bacc(None, target_bir_lowering=False, debug=False)

    xt_d = nc.dram_tensor("xt", [C, N], f32, kind="ExternalInput")
    wt_d = nc.dram_tensor("wt", [C, 3 * C], f32, kind="ExternalInput")
    wpt_d = nc.dram_tensor("wpt", [C, C], f32, kind="ExternalInput")
    bias_d = nc.dram_tensor("bias", [1, C], f32, kind="ExternalInput")
    cose_d = nc.dram_tensor("cose", [P, N], f32, kind="ExternalInput")
    sine_d = nc.dram_tensor("sine", [P, N], f32, kind="ExternalInput")
    emask_d = nc.dram_tensor("emask", [P, NCHUNK], f32, kind="ExternalInput")
    m2t_d = nc.dram_tensor("m2t", [P, P], f32, kind="ExternalInput")
    y_d = nc.dram_tensor("y", [NQ, C], f32, kind="ExternalOutput")

    EXP = mybir.ActivationFunctionType.Exp
    VW = 65  # v columns incl. the emask/ones column

    with tile.TileContext(nc) as tc:
        with (
            tc.tile_pool(name="consts", bufs=1) as consts,
            tc.tile_pool(name="qk", bufs=1) as qk,
            tc.tile_pool(name="vpool", bufs=1) as vpool,
            tc.tile_pool(name="outp", bufs=1) as outp,
            tc.tile_pool(name="expp", bufs=3) as expp,
            tc.tile_pool(name="rrp", bufs=2) as rrp,
            tc.tile_pool(name="ypool", bufs=2) as ypool,
        ):
            # ---- constants / weights ----
            wpt_s = [consts.tile([DH, C], f32, name=f"wpt{i}") for i in range(H)]
            for i in range(H):
                nc.sync.dma_start(wpt_s[i][:], wpt_d.ap()[i * DH:(i + 1) * DH, :])
            bias_s = consts.tile([1, C], f32, name="bias")
            nc.sync.dma_start(bias_s[:], bias_d.ap())
            m2t_s = consts.tile([P, P], f32, name="m2t")
            nc.sync.dma_start(m2t_s[:], m2t_d.ap())
            emask_s = consts.tile([P, NCHUNK], f32, name="emask")
            nc.sync.dma_start(emask_s[:], emask_d.ap())
            ones_s = consts.tile([P, P], f32, name="ones")
            nc.any.memset(ones_s[:], 1.0)

            # ---- persistent activations ----
            qT = [qk.tile([P, NQ], f32, name=f"qT{i}") for i in range(4)]
            kT = [qk.tile([P, N], f32, name=f"kT{i}") for i in range(4)]
            v65 = vpool.tile([P, NCHUNK * H * VW], f32, name="v65")

            # ================= phase 1: qkv projection + RoPE =================
            phase1_cm = tc.tile_pool(name="phase1", bufs=1)
            xts_cm = tc.tile_pool(name="xts", bufs=4)
            tmps_cm = tc.tile_pool(name="tmps", bufs=2)
            cep_cm = tc.tile_pool(name="cep", bufs=2)
            ps1_cm = tc.tile_pool(name="ps1", bufs=2, space="PSUM")
            ps1 = ps1_cm.__enter__()
            phase1 = phase1_cm.__enter__()
            xts = xts_cm.__enter__()
            tmps = tmps_cm.__enter__()
            cep = cep_cm.__enter__()
            wt_s = [phase1.tile([P, 3 * C], f32, name=f"wt{i}") for i in range(4)]
            for i in range(4):
                nc.sync.dma_start(wt_s[i][:], wt_d.ap()[i * P:(i + 1) * P, :])
            for nb in range(4):  # n blocks of 512 key positions
                nbs = slice(nb * 512, (nb + 1) * 512)
                cose_s = cep.tile([P, 512], f32, name="cose")
                nc.sync.dma_start(cose_s[:], cose_d.ap()[:, nbs])
                sine_s = cep.tile([P, 512], f32, name="sine")
                nc.sync.dma_start(sine_s[:], sine_d.ap()[:, nbs])
                xt_t = []
                for ci in range(4):
                    t = xts.tile([P, 512], f32, name="xt_t")
                    nc.sync.dma_start(
                        t[:], xt_d.ap()[ci * P:(ci + 1) * P, nb * 512:(nb + 1) * 512])
                    xt_t.append(t)

                # ---- v projection (natural [n, c] layout) ----
                for tt in range(4):
                    chunk = nb * 4 + tt
                    ps_v = ps1.tile([P, C], f32, name="ps_qkv")
                    for ci in range(4):
                        nc.tensor.matmul(
                            ps_v[:],
                            xt_t[ci][:, tt * P:(tt + 1) * P],
                            wt_s[ci][:, 2 * C:3 * C],
                            start=(ci == 0), stop=(ci == 3))
                    base = chunk * H * VW
                    vv = v65[:, base:base + H * VW].rearrange(
                        "p (h w) -> p h w", w=VW)
                    nc.vector.tensor_scalar_mul(
                        vv[:, :, 0:DH],
                        ps_v[:].rearrange("p (h w) -> p h w", w=DH),
                        emask_s[:, chunk:chunk + 1])
                    nc.vector.tensor_copy(
                        vv[:, :, DH:DH + 1],
                        emask_s[:, chunk:chunk + 1, None].to_broadcast((P, H, 1)))

                # ---- q (only nb<2) and k projections, [c, n] layout + RoPE ----
                species = [("k", C)] if nb >= 2 else [("q", 0), ("k", C)]
                for name_sp, woff in species:
                    for ct in range(4):
                        ps_p = ps1.tile([P, 512], f32, name="ps_qkv")
                        for ci in range(4):
                            nc.tensor.matmul(
                                ps_p[:],
                                wt_s[ci][:, woff + ct * P: woff + (ct + 1) * P],
                                xt_t[ci][:],
                                start=(ci == 0), stop=(ci == 3))
                        t_c = tmps.tile([P, 512], f32, name="t_c")
                        nc.vector.tensor_mul(t_c[:], ps_p[:], cose_s[:])
                        t_s = tmps.tile([P, 512], f32, name="t_s")
                        nc.vector.tensor_mul(t_s[:], ps_p[:], sine_s[:])
                        ps_m2 = ps1.tile([P, 512], f32, name="ps_m2")
                        nc.tensor.matmul(ps_m2[:], m2t_s[:], t_s[:],
                                         start=True, stop=True)
                        dest = qT[ct] if name_sp == "q" else kT[ct]
                        nc.vector.tensor_add(
                            dest[:, nb * 512:(nb + 1) * 512], t_c[:], ps_m2[:])

            cep_cm.__exit__(None, None, None)
            tmps_cm.__exit__(None, None, None)
            xts_cm.__exit__(None, None, None)
            phase1_cm.__exit__(None, None, None)
            ps1_cm.__exit__(None, None, None)

            # ================= phase 2: attention per head =================
            ps_score_cm = tc.tile_pool(name="ps_score", bufs=2, space="PSUM")
            ps_av_cm = tc.tile_pool(name="ps_av", bufs=1, space="PSUM")
            ps_score = ps_score_cm.__enter__()
            ps_av = ps_av_cm.__enter__()
            # per-head attn output scratch [65, 1024]; row 64 = denominators
            sc = [outp.tile([VW, NQ], f32, name=f"sc{h}") for h in range(H)]
            for h in range(H):
                qt, pb = qT[h // 2], (h % 2) * DH
                kt = kT[h // 2]
                ps_o = ps_av.tile([VW, NQ], f32, name="ps_o")
                for c in range(NCHUNK):
                    ps_s = ps_score.tile([P, NQ], f32, name="ps_s")
                    for qb in range(2):
                        nc.tensor.matmul(
                            ps_s[:, qb * 512:(qb + 1) * 512],
                            kt[pb:pb + DH, c * P:(c + 1) * P],
                            qt[pb:pb + DH, qb * 512:(qb + 1) * 512],
                            start=True, stop=True)
                    ex = expp.tile([P, NQ], f32, name="ex")
                    nc.scalar.activation(ex[:], ps_s[:], EXP, scale=0.125)
                    voff = (c * H + h) * VW
                    for qb in range(2):
                        nc.tensor.matmul(
                            ps_o[:, qb * 512:(qb + 1) * 512],
                            v65[:, voff:voff + VW],
                            ex[:, qb * 512:(qb + 1) * 512],
                            start=(c == 0), stop=(c == NCHUNK - 1))
                nc.vector.tensor_copy(sc[h][:], ps_o[:])

            ps_av_cm.__exit__(None, None, None)
            ps_score_cm.__exit__(None, None, None)
            ps3_cm = tc.tile_pool(name="ps3", bufs=2, space="PSUM")
            ps3 = ps3_cm.__enter__()

            # ================= phase 3: normalize + output projection =========
            for h in range(H):
                ps_r = ps3.tile([DH, NQ], f32, name="ps_r")
                for qb in range(2):
                    nc.tensor.matmul(
                        ps_r[:, qb * 512:(qb + 1) * 512],
                        ones_s[DH:DH + 1, 0:DH],
                        sc[h][DH:DH + 1, qb * 512:(qb + 1) * 512],
                        start=True, stop=True)
                rr = rrp.tile([DH, NQ], f32, name="rr")
                nc.vector.reciprocal_approx_fast(rr[:], ps_r[:])
                nc.vector.tensor_mul(sc[h][0:DH, :], sc[h][0:DH, :], rr[:])

            for nbk in range(8):  # output row blocks of 128
                ps_y = ps3.tile([P, C], f32, name="ps_y")
                nc.tensor.matmul(ps_y[:], ones_s[0:1, 0:P], bias_s[:],
                                 start=True, stop=False)
                for h in range(H):
                    nc.tensor.matmul(
                        ps_y[:],
                        sc[h][0:DH, nbk * P:(nbk + 1) * P],
                        wpt_s[h][:],
                        start=False, stop=(h == H - 1))
                y_s = ypool.tile([P, C], f32, name="y_s")
                nc.vector.tensor_copy(y_s[:], ps_y[:])
                nc.sync.dma_start(y_d.ap()[nbk * P:(nbk + 1) * P, :], y_s[:])
            ps3_cm.__exit__(None, None, None)

    nc.compile()
    return nc


def _get_module():
    if "nc" not in _CACHE:
        _CACHE["nc"] = _build_module()
    return _CACHE["nc"]


def kernel(x, mask, times, Wqkv, Wproj, bproj, num_cls_token=0, _trace=False):
    from concourse.bass_utils import run_bass_kernel_spmd

    assert int(num_cls_token) == 0, "kernel specialized for num_cls_token=0"
    in_maps = _host_prep(x, mask, times, Wqkv, Wproj, bproj)
    nc = _get_module()
    res = run_bass_kernel_spmd(nc, in_maps, list(range(8)), trace=_trace)
    _CACHE["last_result"] = res

    out = np.empty((B, N, C), np.float32)
    for core in range(8):
        b, qhalf = core // 2, core % 2
        out[b, qhalf * NQ:(qhalf + 1) * NQ, :] = res.results[core]["y"]
    return out


# revision 11
# speedup vs baseline: 1.8332x; 1.8332x over previous
"""Trainium2 Bass kernel for AttentionWithRotaryPositionalEmbedding.

Problem shapes (hardcoded): x [4, 2048, 512], 8 heads, head dim 64.
Sharding: 8 cores = (batch b = core//2) x (query half = core%2).
Each core computes a [1024, 512] slice of the output; k/v are computed
locally from the full x[b] so no collectives are needed.

Device layout per core (fp32 everywhere):
  - host passes x[b]^T with the core's query half rotated to columns 0:1024
    (a pure permutation of the key axis; attention output is invariant).
  - qT/kT produced in [c, n] layout directly (c on partitions) so scores and
    attn@v need no transposes.
  - RoPE = t_c + M2 @ t_s where t_c/t_s are elementwise cos/sin products and
    M2 is the constant pairwise-rotation permutation (done on PE).
  - scores computed transposed: sT[k, q] = kT_h^T-style matmul; exp on ACT
    with fused *0.125; additive mask folded in exactly as exp(mask) scaling
    of v rows and of the ones-column that produces the softmax denominators.
  - attn@v: lhsT = [v_h | emask] (M=65) accumulating over k chunks; row 64 of
    the psum is the softmax denominator. Normalization is applied after, via
    a PE-replicated fast reciprocal, before the output projection.
"""

import sys

import numpy as np

if "/opt/trn_rl_repo" not in sys.path:
    sys.path.insert(0, "/opt/trn_rl_repo")

B, N, C = 4, 2048, 512
H, DH = 8, 64
NQ = 1024  # queries per core
P = 128
NCHUNK = N // P  # 16 k chunks
MAX_FPS = np.float32(30.0)

_CACHE = {}


def _host_prep(x, mask, times, Wqkv, Wproj, bproj):
    """Build per-core input maps (numpy only)."""
    x = np.asarray(x, np.float32)
    mask = np.asarray(mask, np.float32)
    times = np.asarray(times, np.float32)
    Wqkv = np.asarray(Wqkv, np.float32)
    Wproj = np.asarray(Wproj, np.float32)
    bproj = np.asarray(bproj, np.float32).reshape(1, C)

    wt = np.ascontiguousarray(Wqkv.T)          # [512, 1536] = [WqT|WkT|WvT]
    wpt = np.ascontiguousarray(Wproj.T)        # [512, 512]

    # pairwise rotation permutation: (M2 @ v)[2i] = -v[2i+1]; [2i+1] = +v[2i]
    M2 = np.zeros((P, P), np.float32)
    for i in range(P // 2):
        M2[2 * i, 2 * i + 1] = -1.0
        M2[2 * i + 1, 2 * i] = 1.0
    m2t = np.ascontiguousarray(M2.T)

    # rotary tables (match reference: all f32 math)
    inv_freq = (np.float32(1.0) /
                (np.float32(10000.0) **
                 (np.arange(0, DH, 2, dtype=np.float32) / np.float32(DH))))  # [32]
    pos = np.round(times * MAX_FPS)  # [B, N] f32, round-half-even like jnp

    in_maps = []
    for core in range(8):
        b, qhalf = core // 2, core % 2
        if qhalf == 0:
            perm = np.arange(N)
        else:
            perm = np.r_[NQ:N, 0:NQ]
        xt = np.ascontiguousarray(x[b].T[:, perm])           # [512, 2048]
        freqs = pos[b][perm][None, :] * inv_freq[:, None]     # [32, 2048] f32
        cos32 = np.cos(freqs.astype(np.float32))
        sin32 = np.sin(freqs.astype(np.float32))
        ridx = (np.arange(P) % DH) // 2                       # row -> pair index
        cose = np.ascontiguousarray(cos32[ridx])              # [128, 2048]
        sine = np.ascontiguousarray(sin32[ridx])
        em = np.exp(mask[b][perm]).astype(np.float32)         # [2048]
        emask = np.ascontiguousarray(em.reshape(NCHUNK, P).T) # [128, 16]
        in_maps.append({
            "xt": xt, "wt": wt, "wpt": wpt, "bias": bproj,
            "cose": cose, "sine": sine, "emask": emask, "m2t": m2t,
        })
    return in_maps


def _build_module():
    import concourse.bass as bass
    import concourse.tile as tile
    import concourse.mybir as mybir
    from concourse import bacc

    f32 = mybir.dt.float32
    f32r = mybir.dt.float32r
    nc = bacc.Bacc(None, target_bir_lowering=False, debug=False)

    xt_d = nc.dram_tensor("xt", [C, N], f32r, kind="ExternalInput")
    wt_d = nc.dram_tensor("wt", [C, 3 * C], f32r, kind="ExternalInput")
    wpt_d = nc.dram_tensor("wpt", [C, C], f32r, kind="ExternalInput")
    bias_d = nc.dram_tensor("bias", [1, C], f32r, kind="ExternalInput")
    cose_d = nc.dram_tensor("cose", [P, N], f32, kind="ExternalInput")
    sine_d = nc.dram_tensor("sine", [P, N], f32, kind="ExternalInput")
    emask_d = nc.dram_tensor("emask", [P, NCHUNK], f32, kind="ExternalInput")
    m2t_d = nc.dram_tensor("m2t", [P, P], f32r, kind="ExternalInput")
    y_d = nc.dram_tensor("y", [NQ, C], f32, kind="ExternalOutput")

    EXP = mybir.ActivationFunctionType.Exp
    VW = 65  # v columns incl. the emask/ones column

    with tile.TileContext(nc) as tc:
        with (
            tc.tile_pool(name="consts", bufs=1) as consts,
            tc.tile_pool(name="qk", bufs=1) as qk,
            tc.tile_pool(name="vpool", bufs=1) as vpool,
            tc.tile_pool(name="outp", bufs=1) as outp,
            tc.tile_pool(name="expp", bufs=3) as expp,
            tc.tile_pool(name="rrp", bufs=2) as rrp,
            tc.tile_pool(name="ypool", bufs=2) as ypool,
        ):
            # ---- constants / weights ----
            wpt_s = [consts.tile([DH, C], f32r, name=f"wpt{i}") for i in range(H)]
            for i in range(H):
                nc.sync.dma_start(wpt_s[i][:], wpt_d.ap()[i * DH:(i + 1) * DH, :])
            bias_s = consts.tile([1, C], f32r, name="bias")
            nc.sync.dma_start(bias_s[:], bias_d.ap())
            m2t_s = consts.tile([P, P], f32r, name="m2t")
            nc.sync.dma_start(m2t_s[:], m2t_d.ap())
            emask_s = consts.tile([P, NCHUNK], f32, name="emask")
            nc.sync.dma_start(emask_s[:], emask_d.ap())
            ones_f = consts.tile([P, P], f32, name="ones_f")
            nc.any.memset(ones_f[:], 1.0)
            ones_s = consts.tile([P, P], f32r, name="ones")
            nc.vector.tensor_copy(ones_s[:], ones_f[:])

            # ---- persistent activations ----
            qT = [qk.tile([P, NQ], f32r, name=f"qT{i}") for i in range(4)]
            kT = [qk.tile([P, N], f32r, name=f"kT{i}") for i in range(4)]
            v65 = vpool.tile([P, NCHUNK * H * VW], f32r, name="v65")

            # ================= phase 1: qkv projection + RoPE =================
            phase1_cm = tc.tile_pool(name="phase1", bufs=1)
            xts_cm = tc.tile_pool(name="xts", bufs=4)
            tmps_cm = tc.tile_pool(name="tmps", bufs=2)
            cep_cm = tc.tile_pool(name="cep", bufs=2)
            ps1_cm = tc.tile_pool(name="ps1", bufs=2, space="PSUM")
            ps1 = ps1_cm.__enter__()
            phase1 = phase1_cm.__enter__()
            xts = xts_cm.__enter__()
            tmps = tmps_cm.__enter__()
            cep = cep_cm.__enter__()
            wt_s = [phase1.tile([P, 3 * C], f32r, name=f"wt{i}") for i in range(4)]
            for i in range(4):
                nc.sync.dma_start(wt_s[i][:], wt_d.ap()[i * P:(i + 1) * P, :])
            for nb in range(4):  # n blocks of 512 key positions
                nbs = slice(nb * 512, (nb + 1) * 512)
                cose_s = cep.tile([P, 512], f32, name="cose")
                nc.sync.dma_start(cose_s[:], cose_d.ap()[:, nbs])
                sine_s = cep.tile([P, 512], f32, name="sine")
                nc.sync.dma_start(sine_s[:], sine_d.ap()[:, nbs])
                xt_t = []
                for ci in range(4):
                    t = xts.tile([P, 512], f32r, name="xt_t")
                    nc.sync.dma_start(
                        t[:], xt_d.ap()[ci * P:(ci + 1) * P, nb * 512:(nb + 1) * 512])
                    xt_t.append(t)

                # ---- v projection (natural [n, c] layout) ----
                for tt in range(4):
                    chunk = nb * 4 + tt
                    ps_v = ps1.tile([P, C], f32, name="ps_qkv")
                    for ci in range(4):
                        nc.tensor.matmul(
                            ps_v[:],
                            xt_t[ci][:, tt * P:(tt + 1) * P],
                            wt_s[ci][:, 2 * C:3 * C],
                            start=(ci == 0), stop=(ci == 3))
                    base = chunk * H * VW
                    vv = v65[:, base:base + H * VW].rearrange(
                        "p (h w) -> p h w", w=VW)
                    nc.vector.tensor_scalar_mul(
                        vv[:, :, 0:DH],
                        ps_v[:].rearrange("p (h w) -> p h w", w=DH),
                        emask_s[:, chunk:chunk + 1])
                    nc.vector.tensor_copy(
                        vv[:, :, DH:DH + 1],
                        emask_s[:, chunk:chunk + 1, None].to_broadcast((P, H, 1)))

                # ---- q (only nb<2) and k projections, [c, n] layout + RoPE ----
                species = [("k", C)] if nb >= 2 else [("q", 0), ("k", C)]
                for name_sp, woff in species:
                    for ct in range(4):
                        ps_p = ps1.tile([P, 512], f32, name="ps_qkv")
                        for ci in range(4):
                            nc.tensor.matmul(
                                ps_p[:],
                                wt_s[ci][:, woff + ct * P: woff + (ct + 1) * P],
                                xt_t[ci][:],
                                start=(ci == 0), stop=(ci == 3))
                        t_c = tmps.tile([P, 512], f32, name="t_c")
                        nc.vector.tensor_mul(t_c[:], ps_p[:], cose_s[:])
                        t_s = tmps.tile([P, 512], f32r, name="t_s")
                        nc.vector.tensor_mul(t_s[:], ps_p[:], sine_s[:])
                        ps_m2 = ps1.tile([P, 512], f32, name="ps_m2")
                        nc.tensor.matmul(ps_m2[:], m2t_s[:], t_s[:],
                                         start=True, stop=True)
                        dest = qT[ct] if name_sp == "q" else kT[ct]
                        nc.vector.tensor_add(
                            dest[:, nb * 512:(nb + 1) * 512], t_c[:], ps_m2[:])

            cep_cm.__exit__(None, None, None)
            tmps_cm.__exit__(None, None, None)
            xts_cm.__exit__(None, None, None)
            phase1_cm.__exit__(None, None, None)
            ps1_cm.__exit__(None, None, None)

            # ================= phase 2: attention per head =================
            ps_score_cm = tc.tile_pool(name="ps_score", bufs=2, space="PSUM")
            ps_av_cm = tc.tile_pool(name="ps_av", bufs=1, space="PSUM")
            ps_score = ps_score_cm.__enter__()
            ps_av = ps_av_cm.__enter__()
            # per-head attn output scratch [65, 1024]; row 64 = denominators
            sc = [outp.tile([VW, NQ], f32r, name=f"sc{h}") for h in range(H)]
            for h in range(H):
                qt, pb = qT[h // 2], (h % 2) * DH
                kt = kT[h // 2]
                ps_o = ps_av.tile([VW, NQ], f32, name="ps_o")
                for c in range(NCHUNK):
                    ps_s = ps_score.tile([P, NQ], f32, name="ps_s")
                    for qb in range(2):
                        nc.tensor.matmul(
                            ps_s[:, qb * 512:(qb + 1) * 512],
                            kt[pb:pb + DH, c * P:(c + 1) * P],
                            qt[pb:pb + DH, qb * 512:(qb + 1) * 512],
                            start=True, stop=True)
                    ex = expp.tile([P, NQ], f32r, name="ex")
                    nc.scalar.activation(ex[:], ps_s[:], EXP, scale=0.125)
                    voff = (c * H + h) * VW
                    for qb in range(2):
                        nc.tensor.matmul(
                            ps_o[:, qb * 512:(qb + 1) * 512],
                            v65[:, voff:voff + VW],
                            ex[:, qb * 512:(qb + 1) * 512],
                            start=(c == 0), stop=(c == NCHUNK - 1))
                nc.vector.tensor_copy(sc[h][:], ps_o[:])

            ps_av_cm.__exit__(None, None, None)
            ps_score_cm.__exit__(None, None, None)
            ps3_cm = tc.tile_pool(name="ps3", bufs=2, space="PSUM")
            ps3 = ps3_cm.__enter__()

            # ================= phase 3: normalize + output projection =========
            for h in range(H):
                ps_r = ps3.tile([DH, NQ], f32, name="ps_r")
                for qb in range(2):
                    nc.tensor.matmul(
                        ps_r[:, qb * 512:(qb + 1) * 512],
                        ones_s[DH:DH + 1, 0:DH],
                        sc[h][DH:DH + 1, qb * 512:(qb + 1) * 512],
                        start=True, stop=True)
                rr = rrp.tile([DH, NQ], f32, name="rr")
                nc.vector.reciprocal_approx_fast(rr[:], ps_r[:])
                nc.vector.tensor_mul(sc[h][0:DH, :], sc[h][0:DH, :], rr[:])

            for nbk in range(8):  # output row blocks of 128
                ps_y = ps3.tile([P, C], f32, name="ps_y")
                nc.tensor.matmul(ps_y[:], ones_s[0:1, 0:P], bias_s[:],
                                 start=True, stop=False)
                for h in range(H):
                    nc.tensor.matmul(
                        ps_y[:],
                        sc[h][0:DH, nbk * P:(nbk + 1) * P],
                        wpt_s[h][:],
                        start=False, stop=(h == H - 1))
                y_s = ypool.tile([P, C], f32, name="y_s")
                nc.vector.tensor_copy(y_s[:], ps_y[:])
                nc.sync.dma_start(y_d.ap()[nbk * P:(nbk + 1) * P, :], y_s[:])
            ps3_cm.__exit__(None, None, None)

    nc.compile()
    return nc


def _get_module():
    if "nc" not in _CACHE:
        _CACHE["nc"] = _build_module()
    return _CACHE["nc"]


def kernel(x, mask, times, Wqkv, Wproj, bproj, num_cls_token=0, _trace=False):
    from concourse.bass_utils import run_bass_kernel_spmd

    assert int(num_cls_token) == 0, "kernel specialized for num_cls_token=0"
    in_maps = _host_prep(x, mask, times, Wqkv, Wproj, bproj)
    nc = _get_module()
    res = run_bass_kernel_spmd(nc, in_maps, list(range(8)), trace=_trace)
    _CACHE["last_result"] = res

    out = np.empty((B, N, C), np.float32)
    for core in range(8):
        b, qhalf = core // 2, core % 2
        out[b, qhalf * NQ:(qhalf + 1) * NQ, :] = res.results[core]["y"]
    return out


# revision 13
# speedup vs baseline: 2.1156x; 1.1541x over previous
"""Trainium2 Bass kernel for AttentionWithRotaryPositionalEmbedding.

Problem shapes (hardcoded): x [4, 2048, 512], 8 heads, head dim 64.
Sharding: 8 cores = (batch b = core//2) x (query half = core%2).
Each core computes a [1024, 512] slice of the output; k/v are computed
locally from the full x[b] so no collectives are needed.

Device layout per core (fp32 everywhere):
  - host passes x[b]^T with the core's query half rotated to columns 0:1024
    (a pure permutation of the key axis; attention output is invariant).
  - qT/kT produced in [c, n] layout directly (c on partitions) so scores and
    attn@v need no transposes.
  - RoPE = t_c + M2 @ t_s where t_c/t_s are elementwise cos/sin products and
    M2 is the constant pairwise-rotation permutation (done on PE).
  - scores computed transposed: sT[k, q] = kT_h^T-style matmul; exp on ACT
    with fused *0.125; additive mask folded in exactly as exp(mask) scaling
    of v rows and of the ones-column that produces the softmax denominators.
  - attn@v: lhsT = [v_h | emask] (M=65) accumulating over k chunks; row 64 of
    the psum is the softmax denominator. Normalization is applied after, via
    a PE-replicated fast reciprocal, before the output projection.
"""

import sys

import numpy as np

if "/opt/trn_rl_repo" not in sys.path:
    sys.path.insert(0, "/opt/trn_rl_repo")

B, N, C = 4, 2048, 512
H, DH = 8, 64
NQ = 1024  # queries per core
P = 128
NCHUNK = N // P  # 16 k chunks
MAX_FPS = np.float32(30.0)

_CACHE = {}


def _host_prep(x, mask, times, Wqkv, Wproj, bproj):
    """Build per-core input maps (numpy only)."""
    x = np.asarray(x, np.float32)
    mask = np.asarray(mask, np.float32)
    times = np.asarray(times, np.float32)
    Wqkv = np.asarray(Wqkv, np.float32)
    Wproj = np.asarray(Wproj, np.float32)
    bproj = np.asarray(bproj, np.float32).reshape(1, C)

    wt = np.ascontiguousarray(Wqkv.T)          # [512, 1536] = [WqT|WkT|WvT]
    wpt = np.ascontiguousarray(Wproj.T)        # [512, 512]

    # pairwise rotation permutation: (M2 @ v)[2i] = -v[2i+1]; [2i+1] = +v[2i]
    M2 = np.zeros((P, P), np.float32)
    for i in range(P // 2):
        M2[2 * i, 2 * i + 1] = -1.0
        M2[2 * i + 1, 2 * i] = 1.0
    m2t = np.ascontiguousarray(M2.T)

    # rotary tables (match reference: all f32 math)
    inv_freq = (np.float32(1.0) /
                (np.float32(10000.0) **
                 (np.arange(0, DH, 2, dtype=np.float32) / np.float32(DH))))  # [32]
    pos = np.round(times * MAX_FPS)  # [B, N] f32, round-half-even like jnp

    in_maps = []
    for core in range(8):
        b, qhalf = core // 2, core % 2
        if qhalf == 0:
            perm = np.arange(N)
        else:
            perm = np.r_[NQ:N, 0:NQ]
        xt = np.ascontiguousarray(x[b].T[:, perm])           # [512, 2048]
        freqs = pos[b][perm][None, :] * inv_freq[:, None]     # [32, 2048] f32
        cos32 = np.cos(freqs.astype(np.float32))
        sin32 = np.sin(freqs.astype(np.float32))
        ridx = (np.arange(P) % DH) // 2                       # row -> pair index
        cose = np.ascontiguousarray(cos32[ridx])              # [128, 2048]
        sine = np.ascontiguousarray(sin32[ridx])
        em = np.exp(mask[b][perm]).astype(np.float32)         # [2048]
        emask = np.ascontiguousarray(em.reshape(NCHUNK, P).T) # [128, 16]
        in_maps.append({
            "xt": xt, "wt": wt, "wpt": wpt, "bias": bproj,
            "cose": cose, "sine": sine, "emask": emask, "m2t": m2t,
        })
    return in_maps


def _build_module():
    import concourse.bass as bass
    import concourse.tile as tile
    import concourse.mybir as mybir
    from concourse import bacc

    f32 = mybir.dt.float32
    f32r = mybir.dt.float32r
    bf16 = mybir.dt.bfloat16
    nc = bacc.Bacc(None, target_bir_lowering=False, debug=False)

    xt_d = nc.dram_tensor("xt", [C, N], f32r, kind="ExternalInput")
    wt_d = nc.dram_tensor("wt", [C, 3 * C], f32r, kind="ExternalInput")
    wpt_d = nc.dram_tensor("wpt", [C, C], f32r, kind="ExternalInput")
    bias_d = nc.dram_tensor("bias", [1, C], f32r, kind="ExternalInput")
    cose_d = nc.dram_tensor("cose", [P, N], f32, kind="ExternalInput")
    sine_d = nc.dram_tensor("sine", [P, N], f32, kind="ExternalInput")
    emask_d = nc.dram_tensor("emask", [P, NCHUNK], f32, kind="ExternalInput")
    m2t_d = nc.dram_tensor("m2t", [P, P], f32r, kind="ExternalInput")
    y_d = nc.dram_tensor("y", [NQ, C], f32, kind="ExternalOutput")

    EXP = mybir.ActivationFunctionType.Exp
    VW = 65  # v columns incl. the emask/ones column

    with tile.TileContext(nc) as tc:
        with (
            tc.tile_pool(name="consts", bufs=1) as consts,
            tc.tile_pool(name="qk", bufs=1) as qk,
            tc.tile_pool(name="vpool", bufs=1) as vpool,
            tc.tile_pool(name="outp", bufs=1) as outp,
            tc.tile_pool(name="expp", bufs=3) as expp,
            tc.tile_pool(name="rrp", bufs=2) as rrp,
            tc.tile_pool(name="ypool", bufs=2) as ypool,
        ):
            # ---- constants / weights ----
            wpt_s = [consts.tile([DH, C], f32r, name=f"wpt{i}") for i in range(H)]
            for i in range(H):
                nc.sync.dma_start(wpt_s[i][:], wpt_d.ap()[i * DH:(i + 1) * DH, :])
            bias_s = consts.tile([1, C], f32r, name="bias")
            nc.sync.dma_start(bias_s[:], bias_d.ap())
            m2t_s = consts.tile([P, P], f32r, name="m2t")
            nc.sync.dma_start(m2t_s[:], m2t_d.ap())
            emask_s = consts.tile([P, NCHUNK], f32, name="emask")
            nc.sync.dma_start(emask_s[:], emask_d.ap())
            ones_f = consts.tile([P, P], f32, name="ones_f")
            nc.any.memset(ones_f[:], 1.0)
            ones_s = consts.tile([P, P], f32r, name="ones")
            nc.vector.tensor_copy(ones_s[:], ones_f[:])

            # ---- persistent activations ----
            qT = [qk.tile([P, NQ], bf16, name=f"qT{i}") for i in range(4)]
            kT = [qk.tile([P, N], bf16, name=f"kT{i}") for i in range(4)]
            v65 = vpool.tile([P, NCHUNK * H * VW + 63], f32r, name="v65")
            nc.vector.tensor_scalar_mul(
                v65[:, NCHUNK * H * VW:], ones_f[:, 0:63], 0.0)

            # ================= phase 1: qkv projection + RoPE =================
            phase1_cm = tc.tile_pool(name="phase1", bufs=1)
            xts_cm = tc.tile_pool(name="xts", bufs=4)
            tmps_cm = tc.tile_pool(name="tmps", bufs=2)
            cep_cm = tc.tile_pool(name="cep", bufs=2)
            ps1_cm = tc.tile_pool(name="ps1", bufs=2, space="PSUM")
            ps1 = ps1_cm.__enter__()
            phase1 = phase1_cm.__enter__()
            xts = xts_cm.__enter__()
            tmps = tmps_cm.__enter__()
            cep = cep_cm.__enter__()
            wt_s = [phase1.tile([P, 3 * C], f32r, name=f"wt{i}") for i in range(4)]
            for i in range(4):
                nc.sync.dma_start(wt_s[i][:], wt_d.ap()[i * P:(i + 1) * P, :])
            for nb in range(4):  # n blocks of 512 key positions
                nbs = slice(nb * 512, (nb + 1) * 512)
                cose_s = cep.tile([P, 512], f32, name="cose")
                nc.sync.dma_start(cose_s[:], cose_d.ap()[:, nbs])
                sine_s = cep.tile([P, 512], f32, name="sine")
                nc.sync.dma_start(sine_s[:], sine_d.ap()[:, nbs])
                xt_t = []
                for ci in range(4):
                    t = xts.tile([P, 512], f32r, name="xt_t")
                    nc.sync.dma_start(
                        t[:], xt_d.ap()[ci * P:(ci + 1) * P, nb * 512:(nb + 1) * 512])
                    xt_t.append(t)

                # ---- v projection (natural [n, c] layout) ----
                for tt in range(4):
                    chunk = nb * 4 + tt
                    ps_v = ps1.tile([P, C], f32, name="ps_qkv")
                    for ci in range(4):
                        nc.tensor.matmul(
                            ps_v[:],
                            xt_t[ci][:, tt * P:(tt + 1) * P],
                            wt_s[ci][:, 2 * C:3 * C],
                            start=(ci == 0), stop=(ci == 3))
                    base = chunk * H * VW
                    vv = v65[:, base:base + H * VW].rearrange(
                        "p (h w) -> p h w", w=VW)
                    nc.vector.tensor_scalar_mul(
                        vv[:, :, 0:DH],
                        ps_v[:].rearrange("p (h w) -> p h w", w=DH),
                        emask_s[:, chunk:chunk + 1])
                    nc.vector.tensor_copy(
                        vv[:, :, DH:DH + 1],
                        emask_s[:, chunk:chunk + 1, None].to_broadcast((P, H, 1)))

                # ---- q (only nb<2) and k projections, [c, n] layout + RoPE ----
                species = [("k", C)] if nb >= 2 else [("q", 0), ("k", C)]
                for name_sp, woff in species:
                    for ct in range(4):
                        ps_p = ps1.tile([P, 512], f32, name="ps_qkv")
                        for ci in range(4):
                            nc.tensor.matmul(
                                ps_p[:],
                                wt_s[ci][:, woff + ct * P: woff + (ct + 1) * P],
                                xt_t[ci][:],
                                start=(ci == 0), stop=(ci == 3))
                        t_c = tmps.tile([P, 512], f32, name="t_c")
                        nc.vector.tensor_mul(t_c[:], ps_p[:], cose_s[:])
                        t_s = tmps.tile([P, 512], f32r, name="t_s")
                        nc.vector.tensor_mul(t_s[:], ps_p[:], sine_s[:])
                        ps_m2 = ps1.tile([P, 512], f32, name="ps_m2")
                        nc.tensor.matmul(ps_m2[:], m2t_s[:], t_s[:],
                                         start=True, stop=True)
                        dest = qT[ct] if name_sp == "q" else kT[ct]
                        nc.vector.tensor_add(
                            dest[:, nb * 512:(nb + 1) * 512], t_c[:], ps_m2[:])

            cep_cm.__exit__(None, None, None)
            tmps_cm.__exit__(None, None, None)
            xts_cm.__exit__(None, None, None)
            phase1_cm.__exit__(None, None, None)
            ps1_cm.__exit__(None, None, None)

            # ================= phase 2: attention per head =================
            ps_score_cm = tc.tile_pool(name="ps_score", bufs=2, space="PSUM")
            ps_av_cm = tc.tile_pool(name="ps_av", bufs=1, space="PSUM")
            ps_score = ps_score_cm.__enter__()
            ps_av = ps_av_cm.__enter__()
            # per-head attn output scratch [65, 1024]; row 64 = denominators
            sc = [outp.tile([VW, NQ], f32r, name=f"sc{h}") for h in range(H)]
            for h in range(H):
                qt, pb = qT[h // 2], (h % 2) * DH
                kt = kT[h // 2]
                ps_o = ps_av.tile([P, NQ], f32, name="ps_o")
                for c in range(NCHUNK):
                    ps_s = ps_score.tile([P, NQ], f32, name="ps_s")
                    for qb in range(2):
                        nc.tensor.matmul(
                            ps_s[:, qb * 512:(qb + 1) * 512],
                            kt[pb:pb + DH, c * P:(c + 1) * P],
                            qt[pb:pb + DH, qb * 512:(qb + 1) * 512],
                            start=True, stop=True)
                    ex = expp.tile([P, NQ], f32r, name="ex")
                    nc.scalar.activation(ex[:], ps_s[:], EXP, scale=0.125)
                    voff = (c * H + h) * VW
                    for qb in range(2):
                        nc.tensor.matmul(
                            ps_o[:, qb * 512:(qb + 1) * 512],
                            v65[:, voff:voff + P],
                            ex[:, qb * 512:(qb + 1) * 512],
                            start=(c == 0), stop=(c == NCHUNK - 1))
                nc.vector.tensor_copy(sc[h][:], ps_o[0:VW, :])

            ps_av_cm.__exit__(None, None, None)
            ps_score_cm.__exit__(None, None, None)
            ps3_cm = tc.tile_pool(name="ps3", bufs=2, space="PSUM")
            ps3 = ps3_cm.__enter__()

            # ================= phase 3: normalize + output projection =========
            for h in range(H):
                ps_r = ps3.tile([DH, NQ], f32, name="ps_r")
                for qb in range(2):
                    nc.tensor.matmul(
                        ps_r[:, qb * 512:(qb + 1) * 512],
                        ones_s[DH:DH + 1, 0:DH],
                        sc[h][DH:DH + 1, qb * 512:(qb + 1) * 512],
                        start=True, stop=True)
                rr = rrp.tile([DH, NQ], f32, name="rr")
                nc.vector.reciprocal_approx_fast(rr[:], ps_r[:])
                nc.vector.tensor_mul(sc[h][0:DH, :], sc[h][0:DH, :], rr[:])

            for nbk in range(8):  # output row blocks of 128
                ps_y = ps3.tile([P, C], f32, name="ps_y")
                nc.tensor.matmul(ps_y[:], ones_s[0:1, 0:P], bias_s[:],
                                 start=True, stop=False)
                for h in range(H):
                    nc.tensor.matmul(
                        ps_y[:],
                        sc[h][0:DH, nbk * P:(nbk + 1) * P],
                        wpt_s[h][:],
                        start=False, stop=(h == H - 1))
                y_s = ypool.tile([P, C], f32, name="y_s")
                nc.vector.tensor_copy(y_s[:], ps_y[:])
                nc.sync.dma_start(y_d.ap()[nbk * P:(nbk + 1) * P, :], y_s[:])
            ps3_cm.__exit__(None, None, None)

    nc.compile()
    return nc


def _get_module():
    if "nc" not in _CACHE:
        _CACHE["nc"] = _build_module()
    return _CACHE["nc"]


def kernel(x, mask, times, Wqkv, Wproj, bproj, num_cls_token=0, _trace=False):
    from concourse.bass_utils import run_bass_kernel_spmd

    assert int(num_cls_token) == 0, "kernel specialized for num_cls_token=0"
    in_maps = _host_prep(x, mask, times, Wqkv, Wproj, bproj)
    nc = _get_module()
    res = run_bass_kernel_spmd(nc, in_maps, list(range(8)), trace=_trace)
    _CACHE["last_result"] = res

    out = np.empty((B, N, C), np.float32)
    for core in range(8):
        b, qhalf = core // 2, core % 2
        out[b, qhalf * NQ:(qhalf + 1) * NQ, :] = res.results[core]["y"]
    return out
